# revision 29
# baseline (speedup 1.0000x reference)
"""GAE (Generalized Advantage Estimation) Bass kernel for 8 Trainium2 cores.

Problem: rewards (2048, 8192) f32, values (2048, 8192) f32,
next_values (2048,) f32.
  next_v[:, t] = values[:, t+1] (t < S-1), next_values (t = S-1)
  deltas = rewards + GAMMA * next_v - values  (B, S)
  A_t = deltas_t + (GAMMA*LAM) * A_{t+1}   (A_S = 0, backward recurrence)
  advantages = A, returns = A + values

Sharding: pure data parallel over the batch dim — 2048 rows / 8 cores =
256 rows per core; the seq recurrence is row-local so there is no
cross-core communication.

All DRAM I/O is bf16 (tolerance 2e-2; bf16 round-trip measures ~6e-3),
halving HBM traffic per core to 16.8MB — the f32 kernel was pinned at
the per-core DMA roofline. tensor_tensor_scan keeps an fp32 internal
state regardless of operand dtype, so the recurrence loses no
precision.

Returns satisfy their own backward recurrence, which needs one fewer
elementwise pass than the advantages form:
  B_t = e_t + c*B_{t+1},  e_t = r_t + g*v_{t+1},  c = gamma*lam,
  g = gamma*(1-lam),  B_S = nv;  returns = B, advantages = B - v.

Layout: the whole per-core working set fits in SBUF (v, r, ret tiles =
96KB of the 208KB per partition), so there is no chunking and no
chunk-boundary edge handling — v_{t+1} is always a plain shifted slice.
The terminal edge folds into the scan initial: B_{S-1} = r_{S-1} +
c*(nv/lam), so the host pre-scales next_values by 1/lam and the kernel
has zero edge-column ops (one 1-col scan produces ret[:, S-1]).

Engine split (lessons from NTFF traces of earlier revisions):
  ACT   seeds each PSUM piece with r (copy, bf16->f32), plus the tiny
        nv/weight loads on its DGE queue.
  PE    accumulates (g*I) @ v_shift on top (start=False,
        skip_group_check — the group opens with an engine write). The
        scan reads e straight out of PSUM — no copy-back pass. One
        matmul pass instead of two keeps PE (~2.8us/piece) well under
        DVE's ~5.2us/piece cadence.
  DVE   scan 2048-col pieces (2cyc/col) + the low half of each
        adv = ret - v (bf16 2x_1p). ~44us busy, the pacer.
  Pool  high half of each subtract (~2.5us/piece at 35% duty — at this
        duty cycle it does NOT stretch concurrent DVE ops; a
        full-time Pool measured 2-3.6x DVE stretching earlier).
  DMA   loads then stores ride the sync ring in issue order; loads are
        all issued first so a store waiting on compute can't
        head-of-line-block a load (that cost ~25us in one revision).
"""

import sys

if "/opt/trn_rl_repo" not in sys.path:
    sys.path.insert(0, "/opt/trn_rl_repo")

import numpy as np

GAMMA = 0.99
LAM = 0.95
C_COEF = GAMMA * LAM

B, S = 2048, 8192
N_CORES = 8
ROWS = B // N_CORES  # 256 rows per core
P = 128  # SBUF partitions
N_TILES = ROWS // P  # 2 row-tiles per core
PIECE = 2048  # scan granularity; PSUM holds 2 pieces (4 banks each)
MM = 512  # matmul moving-operand limit

_CACHE: dict = {}


def _build():
    import concourse.bacc as bacc
    import concourse.mybir as mybir
    from concourse.tile import TileContext

    f32 = mybir.dt.float32
    bf16 = mybir.dt.bfloat16
    add = mybir.AluOpType.add
    sub = mybir.AluOpType.subtract
    mult = mybir.AluOpType.mult

    g1ml = GAMMA * (1.0 - LAM)
    nc = bacc.Bacc("TRN2", target_bir_lowering=False, name="gae8")
    r = nc.dram_tensor("rewards", [ROWS, S], bf16, kind="ExternalInput")
    v = nc.dram_tensor("values", [ROWS, S], bf16, kind="ExternalInput")
    # next_values, pre-scaled by 1/lam on the host (see module docstring)
    nv = nc.dram_tensor("next_values", [ROWS], f32, kind="ExternalInput")
    # identity and g*identity weight matrices for the PE e-build
    ident = nc.dram_tensor("ident", [P, P], bf16, kind="ExternalInput")
    gident = nc.dram_tensor("gident", [P, P], bf16, kind="ExternalInput")
    adv = nc.dram_tensor("adv", [ROWS, S], bf16, kind="ExternalOutput")
    ret = nc.dram_tensor("ret", [ROWS, S], bf16, kind="ExternalOutput")

    with TileContext(nc) as tc:
        with (
            tc.tile_pool(name="sb", bufs=1) as sb,
            tc.tile_pool(name="psum", bufs=2, space="PSUM") as psum,
        ):
            c_t = sb.tile([P, 1], f32)
            i_t = sb.tile([P, P], bf16)
            gi_t = sb.tile([P, P], bf16)
            nvc = [
                sb.tile([P, 1], f32, name=f"nvc{t}", tag=f"nvc{t}")
                for t in range(N_TILES)
            ]
            v_t = [
                sb.tile([P, S], bf16, name=f"v{t}", tag=f"v{t}")
                for t in range(N_TILES)
            ]
            r_t = [
                sb.tile([P, S], bf16, name=f"r{t}", tag=f"r{t}")
                for t in range(N_TILES)
            ]
            ret_t = [
                sb.tile([P, S], bf16, name=f"ret{t}", tag=f"ret{t}")
                for t in range(N_TILES)
            ]

            nc.vector.memset(c_t[:, :], C_COEF)
            # The small weight/nv loads are descriptor-heavy (128 tiny
            # descriptors each, ~0.7us); they all ride the scalar ring —
            # its queue is otherwise idle at the start, and the sync ring
            # streams the first compute piece immediately. nv first: the
            # edge scan needs it before PE needs weights.
            for t in range(N_TILES):
                nc.scalar.dma_start(
                    out=nvc[t][:, :],
                    in_=nv[t * P : (t + 1) * P].unsqueeze(1),
                )
            nc.scalar.dma_start(out=i_t[:, :], in_=ident[:, :])
            nc.scalar.dma_start(out=gi_t[:, :], in_=gident[:, :])
            # Piece schedule: tile 0 leads with a small 512-col piece so the
            # scan chain starts as early as possible (the edge scan needs
            # only r's rightmost columns + nv), tile 1 runs uniform pieces.
            pieces = {
                0: [(7680, 8192), (6656, 7680), (4608, 6656), (2560, 4608),
                    (512, 2560), (0, 512)],
                1: [(6144, 8192), (4096, 6144), (2048, 4096), (0, 2048)],
            }
            # All loads piece-by-piece right-to-left, r before v (the edge
            # scan + e-build consume r first); tile 1's pieces follow tile
            # 0's so its first piece lands well before the scan chain gets
            # there (a monolithic tile-1 load measured an 8.8us DVE stall).
            for t in range(N_TILES):
                rows = slice(t * P, (t + 1) * P)
                for p0, p1 in pieces[t]:
                    cs = slice(p0, p1)
                    nc.sync.dma_start(out=r_t[t][:, cs], in_=r[rows, cs])
                    nc.sync.dma_start(out=v_t[t][:, cs], in_=v[rows, cs])

            for t in range(N_TILES):
                rows = slice(t * P, (t + 1) * P)
                # ret[:, S-1] = r[:, S-1] + c*(nv/lam) = r + gamma*nv
                nc.vector.tensor_tensor_scan(
                    out=ret_t[t][:, S - 1 : S],
                    data0=c_t[:, :],
                    data1=r_t[t][:, S - 1 : S],
                    initial=nvc[t][:, 0:1],
                    op0=mult,
                    op1=add,
                )
                for pi, (p0, p1) in enumerate(pieces[t]):
                    # e columns [p0, p0+w); the tile's last column is done
                    # (edge scan above), interior pieces cover full width
                    w = (p1 - p0) - (1 if pi == 0 else 0)
                    if t == 0 and pi == 0:
                        # Kernel-start latency cut: the very first (small)
                        # piece builds e on DVE directly (one stt) instead
                        # of the load->ACT copy->PE matmul->scan chain —
                        # two fewer cross-engine sem hops before the scan
                        # chain starts.
                        e0 = sb.tile([P, w], f32, name="e0", tag="e0")
                        nc.vector.scalar_tensor_tensor(
                            out=e0[:, 0:w],
                            in0=v_t[t][:, p0 + 1 : p0 + w + 1],
                            scalar=g1ml,
                            in1=r_t[t][:, p0 : p0 + w],
                            op0=mult,
                            op1=add,
                        )
                        data1 = e0[:, 0:w][:, ::-1]
                    else:
                        eps = psum.tile([P, PIECE], f32)
                        # e = I @ r + (g*I) @ v_shift accumulated in PSUM,
                        # one (start, stop) matmul pair per 512-col bank.
                        # (An ACT copy seeding PSUM with r, halving PE work,
                        # measured faster but RACES on hardware: the ACT
                        # completion semaphore fires before its posted PSUM
                        # writes are visible to the PE's read-modify-write
                        # accumulate — intermittent corruption. PE-only
                        # accumulation is ordered by construction.)
                        for j in range(0, w, MM):
                            jw = min(MM, w - j)
                            nc.tensor.matmul(
                                eps[:, j : j + jw],
                                i_t[:, :],
                                r_t[t][:, p0 + j : p0 + j + jw],
                                start=True,
                                stop=False,
                            )
                            nc.tensor.matmul(
                                eps[:, j : j + jw],
                                gi_t[:, :],
                                v_t[t][:, p0 + j + 1 : p0 + j + jw + 1],
                                start=False,
                                stop=True,
                            )
                        data1 = eps[:, 0:w][:, ::-1]
                    # backward recurrence over reversed views, fp32 state,
                    # data1 straight from PSUM
                    nc.vector.tensor_tensor_scan(
                        out=ret_t[t][:, p0 : p0 + w][:, ::-1],
                        data0=c_t[:, :].broadcast_to([P, w]),
                        data1=data1,
                        initial=ret_t[t][:, p0 + w : p0 + w + 1],
                        op0=mult,
                        op1=add,
                    )
                    # advantages = returns - v into the freed r slots;
                    # covers the edge col too. Interior pieces split the
                    # subtract between Pool (high half, TensorTensor at
                    # ~2.4ns/col but off the critical DVE stream) and DVE
                    # (low half, bf16 2x_1p 0.6ns/col) — this shaves ~5us
                    # off the DVE body. The globally last piece stays all
                    # on DVE, split in halves, so the tail drains fast.
                    pw = p1 - p0
                    last_piece = t == N_TILES - 1 and p0 == 0
                    if last_piece:
                        parts = [
                            ("v", p0 + pw // 2, p1, True),
                            ("v", p0, p0 + pw // 2, True),
                        ]
                    elif pw >= 1024:
                        # Pool takes the whole interior subtract (~4.9us at
                        # 2.4ns/col, inside PE's ~5.5us piece cadence): DVE
                        # then runs scans only and PE paces the body
                        parts = [("g", p0, p1, True)]
                    else:
                        parts = [("v", p0, p1, True)]
                    for eng, h0, h1, do_store in parts:
                        op_eng = nc.gpsimd if eng == "g" else nc.vector
                        op_eng.tensor_tensor(
                            out=r_t[t][:, h0:h1],
                            in0=ret_t[t][:, h0:h1],
                            in1=v_t[t][:, h0:h1],
                            op=sub,
                        )
                    cs = slice(p0, p1)
                    nc.sync.dma_start(out=ret[rows, cs], in_=ret_t[t][:, cs])
                    if last_piece:
                        nc.sync.dma_start(
                            out=adv[rows, p0 + pw // 2 : p1],
                            in_=r_t[t][:, p0 + pw // 2 : p1],
                        )
                        nc.sync.dma_start(
                            out=adv[rows, p0 : p0 + pw // 2],
                            in_=r_t[t][:, p0 : p0 + pw // 2],
                        )
                    else:
                        nc.sync.dma_start(out=adv[rows, cs], in_=r_t[t][:, cs])
    nc.finalize()
    return nc


def _get_nc():
    if "nc" not in _CACHE:
        _CACHE["nc"] = _build()
    return _CACHE["nc"]


def _run(rewards, values, next_values, **spmd_kwargs):
    """Shard over cores, run the Bass kernel, return BassKernelResults."""
    import ml_dtypes

    from concourse.bass_utils import run_bass_kernel_spmd

    bf16 = ml_dtypes.bfloat16
    nc = _get_nc()
    rewards = np.ascontiguousarray(rewards).astype(bf16)
    values = np.ascontiguousarray(values).astype(bf16)
    # B_{S-1} = r + c*(nv/lam) = r + gamma*nv: pre-scale so the kernel's
    # scan initial needs no edge handling
    nvs = np.ascontiguousarray(next_values, dtype=np.float32) / np.float32(LAM)
    ident = np.eye(P, dtype=bf16)
    gident = (np.eye(P) * (GAMMA * (1.0 - LAM))).astype(bf16)
    in_maps = []
    for c in range(N_CORES):
        sl = slice(c * ROWS, (c + 1) * ROWS)
        in_maps.append(
            {
                "rewards": rewards[sl],
                "values": values[sl],
                "next_values": nvs[sl],
                "ident": ident,
                "gident": gident,
            }
        )
    return run_bass_kernel_spmd(
        nc, in_maps, core_ids=list(range(N_CORES)), **spmd_kwargs
    )


def kernel(rewards, values, next_values):
    res = _run(rewards, values, next_values)
    advantages = np.concatenate(
        [res.results[c]["adv"] for c in range(N_CORES)], 0
    ).astype(np.float32)
    returns = np.concatenate(
        [res.results[c]["ret"] for c in range(N_CORES)], 0
    ).astype(np.float32)
    return advantages, returns


# revision 31
# speedup vs baseline: 1.0392x; 1.0392x over previous
"""GAE (Generalized Advantage Estimation) Bass kernel for 8 Trainium2 cores.

Problem: rewards (2048, 8192) f32, values (2048, 8192) f32,
next_values (2048,) f32.
  next_v[:, t] = values[:, t+1] (t < S-1), next_values (t = S-1)
  deltas = rewards + GAMMA * next_v - values  (B, S)
  A_t = deltas_t + (GAMMA*LAM) * A_{t+1}   (A_S = 0, backward recurrence)
  advantages = A, returns = A + values

Sharding: pure data parallel over the batch dim — 2048 rows / 8 cores =
256 rows per core; the seq recurrence is row-local so there is no
cross-core communication.

All DRAM I/O is bf16 (tolerance 2e-2; bf16 round-trip measures ~6e-3),
halving HBM traffic per core to 16.8MB — the f32 kernel was pinned at
the per-core DMA roofline. tensor_tensor_scan keeps an fp32 internal
state regardless of operand dtype, so the recurrence loses no
precision.

Returns satisfy their own backward recurrence, which needs one fewer
elementwise pass than the advantages form:
  B_t = e_t + c*B_{t+1},  e_t = r_t + g*v_{t+1},  c = gamma*lam,
  g = gamma*(1-lam),  B_S = nv;  returns = B, advantages = B - v.

Layout: the whole per-core working set fits in SBUF (v, r, ret tiles =
96KB of the 208KB per partition), so there is no chunking and no
chunk-boundary edge handling — v_{t+1} is always a plain shifted slice.
The terminal edge folds into the scan initial: B_{S-1} = r_{S-1} +
c*(nv/lam), so the host pre-scales next_values by 1/lam and the kernel
has zero edge-column ops (one 1-col scan produces ret[:, S-1]).

Engine split (lessons from NTFF traces of earlier revisions):
  PE    builds e = I@r + (g*I)@v_shift per 2048-col piece, one
        (start, stop) matmul pair per 512-col PSUM bank; identity
        weight matrices ship from the host. The scan reads e straight
        out of PSUM — no copy-back pass. (An ACT copy seeding PSUM
        with r halved PE work and measured ~4us faster, but RACES on
        hardware — the ACT completion semaphore fires before its
        posted PSUM writes are visible to the PE's read-modify-write
        accumulate; intermittent corruption. PE-only accumulation is
        ordered by construction.) ~35us busy.
  DVE   the scans only: 1-col edge scan + piece scans (2cyc/col,
        irreducible — TensorTensorScanArith has no 16-bit fast mode),
        plus the first piece's e via one stt to skip the PSUM latency
        chain at startup, plus the last piece's subtract (tail). ~40us
        busy, the pacer.
  Pool  the whole interior adv = ret - v (TensorTensor subtract,
        ~2.4ns/col, ~4.9us/piece inside the ~5.5us piece cadence). At
        this duty cycle it does not stretch concurrent DVE ops; a
        full-time Pool alongside a busy DVE measured 2-3.6x DVE
        stretching in an earlier revision.
  ACT   only the tiny descriptor-heavy nv/weight loads on its DGE
        queue (they cost ~3us and must not delay the big loads).
  DMA   loads then stores ride the sync ring in issue order; loads are
        all issued first so a store waiting on compute can't
        head-of-line-block a load (that cost ~25us in one revision).
        Tile-0 loads land piece-by-piece, r before v, so the scan
        chain starts ~10us in (incl. the ~7us NEFF preamble).

Measured: 61.0us/core (HW exec, max over 8 cores), rel err 5.1e-3,
bit-identical across repeated runs. Baseline f32 kernel: 98.9-110.9us.
"""

import sys

if "/opt/trn_rl_repo" not in sys.path:
    sys.path.insert(0, "/opt/trn_rl_repo")

import numpy as np

GAMMA = 0.99
LAM = 0.95
C_COEF = GAMMA * LAM

B, S = 2048, 8192
N_CORES = 8
ROWS = B // N_CORES  # 256 rows per core
P = 128  # SBUF partitions
N_TILES = ROWS // P  # 2 row-tiles per core
PIECE = 2048  # scan granularity; PSUM holds 2 pieces (4 banks each)
MM = 512  # matmul moving-operand limit

_CACHE: dict = {}


def _build():
    import concourse.bacc as bacc
    import concourse.mybir as mybir
    from concourse.tile import TileContext

    f32 = mybir.dt.float32
    bf16 = mybir.dt.bfloat16
    add = mybir.AluOpType.add
    sub = mybir.AluOpType.subtract
    mult = mybir.AluOpType.mult

    g1ml = GAMMA * (1.0 - LAM)
    nc = bacc.Bacc("TRN2", target_bir_lowering=False, name="gae8")
    r = nc.dram_tensor("rewards", [ROWS, S], bf16, kind="ExternalInput")
    v = nc.dram_tensor("values", [ROWS, S], bf16, kind="ExternalInput")
    # next_values, pre-scaled by 1/lam on the host (see module docstring)
    nv = nc.dram_tensor("next_values", [ROWS], f32, kind="ExternalInput")
    # identity and g*identity weight matrices for the PE e-build
    ident = nc.dram_tensor("ident", [P, P], bf16, kind="ExternalInput")
    gident = nc.dram_tensor("gident", [P, P], bf16, kind="ExternalInput")
    adv = nc.dram_tensor("adv", [ROWS, S], bf16, kind="ExternalOutput")
    ret = nc.dram_tensor("ret", [ROWS, S], bf16, kind="ExternalOutput")

    with TileContext(nc) as tc:
        with (
            tc.tile_pool(name="sb", bufs=1) as sb,
            tc.tile_pool(name="psum", bufs=2, space="PSUM") as psum,
        ):
            c_t = sb.tile([P, 1], f32)
            i_t = sb.tile([P, P], bf16)
            gi_t = sb.tile([P, P], bf16)
            nvc = [
                sb.tile([P, 1], f32, name=f"nvc{t}", tag=f"nvc{t}")
                for t in range(N_TILES)
            ]
            v_t = [
                sb.tile([P, S], bf16, name=f"v{t}", tag=f"v{t}")
                for t in range(N_TILES)
            ]
            r_t = [
                sb.tile([P, S], bf16, name=f"r{t}", tag=f"r{t}")
                for t in range(N_TILES)
            ]
            ret_t = [
                sb.tile([P, S], bf16, name=f"ret{t}", tag=f"ret{t}")
                for t in range(N_TILES)
            ]

            nc.vector.memset(c_t[:, :], C_COEF)
            # The small weight/nv loads are descriptor-heavy (128 tiny
            # descriptors each, ~0.7us); they all ride the scalar ring —
            # its queue is otherwise idle at the start, and the sync ring
            # streams the first compute piece immediately. nv first: the
            # edge scan needs it before PE needs weights.
            for t in range(N_TILES):
                nc.scalar.dma_start(
                    out=nvc[t][:, :],
                    in_=nv[t * P : (t + 1) * P].unsqueeze(1),
                )
            nc.scalar.dma_start(out=i_t[:, :], in_=ident[:, :])
            nc.scalar.dma_start(out=gi_t[:, :], in_=gident[:, :])
            # Piece schedule: tile 0 leads with a small 512-col piece so the
            # scan chain starts as early as possible (the edge scan needs
            # only r's rightmost columns + nv), tile 1 runs uniform pieces.
            pieces = {
                0: [(7680, 8192), (5632, 7680), (3584, 5632), (1536, 3584),
                    (0, 1536)],
                1: [(6144, 8192), (4096, 6144), (2048, 4096), (0, 2048)],
            }
            # All loads piece-by-piece right-to-left, r before v (the edge
            # scan + e-build consume r first); tile 1's pieces follow tile
            # 0's so its first piece lands well before the scan chain gets
            # there (a monolithic tile-1 load measured an 8.8us DVE stall).
            for t in range(N_TILES):
                rows = slice(t * P, (t + 1) * P)
                for p0, p1 in pieces[t]:
                    cs = slice(p0, p1)
                    nc.sync.dma_start(out=r_t[t][:, cs], in_=r[rows, cs])
                    nc.sync.dma_start(out=v_t[t][:, cs], in_=v[rows, cs])

            for t in range(N_TILES):
                rows = slice(t * P, (t + 1) * P)
                # ret[:, S-1] = r[:, S-1] + c*(nv/lam) = r + gamma*nv
                nc.vector.tensor_tensor_scan(
                    out=ret_t[t][:, S - 1 : S],
                    data0=c_t[:, :],
                    data1=r_t[t][:, S - 1 : S],
                    initial=nvc[t][:, 0:1],
                    op0=mult,
                    op1=add,
                )
                for pi, (p0, p1) in enumerate(pieces[t]):
                    # e columns [p0, p0+w); the tile's last column is done
                    # (edge scan above), interior pieces cover full width
                    w = (p1 - p0) - (1 if pi == 0 else 0)
                    if t == 0 and pi == 0:
                        # Kernel-start latency cut: the very first (small)
                        # piece builds e on DVE directly (one stt) instead
                        # of the load->ACT copy->PE matmul->scan chain —
                        # two fewer cross-engine sem hops before the scan
                        # chain starts.
                        e0 = sb.tile([P, w], f32, name="e0", tag="e0")
                        nc.vector.scalar_tensor_tensor(
                            out=e0[:, 0:w],
                            in0=v_t[t][:, p0 + 1 : p0 + w + 1],
                            scalar=g1ml,
                            in1=r_t[t][:, p0 : p0 + w],
                            op0=mult,
                            op1=add,
                        )
                        data1 = e0[:, 0:w][:, ::-1]
                    else:
                        eps = psum.tile([P, PIECE], f32)
                        # e = I @ r + (g*I) @ v_shift accumulated in PSUM,
                        # one (start, stop) matmul pair per 512-col bank.
                        # (An ACT copy seeding PSUM with r, halving PE work,
                        # measured faster but RACES on hardware: the ACT
                        # completion semaphore fires before its posted PSUM
                        # writes are visible to the PE's read-modify-write
                        # accumulate — intermittent corruption. PE-only
                        # accumulation is ordered by construction.)
                        for j in range(0, w, MM):
                            jw = min(MM, w - j)
                            nc.tensor.matmul(
                                eps[:, j : j + jw],
                                i_t[:, :],
                                r_t[t][:, p0 + j : p0 + j + jw],
                                start=True,
                                stop=False,
                            )
                            nc.tensor.matmul(
                                eps[:, j : j + jw],
                                gi_t[:, :],
                                v_t[t][:, p0 + j + 1 : p0 + j + jw + 1],
                                start=False,
                                stop=True,
                            )
                        data1 = eps[:, 0:w][:, ::-1]
                    # backward recurrence over reversed views, fp32 state,
                    # data1 straight from PSUM
                    nc.vector.tensor_tensor_scan(
                        out=ret_t[t][:, p0 : p0 + w][:, ::-1],
                        data0=c_t[:, :].broadcast_to([P, w]),
                        data1=data1,
                        initial=ret_t[t][:, p0 + w : p0 + w + 1],
                        op0=mult,
                        op1=add,
                    )
                    # advantages = returns - v into the freed r slots;
                    # covers the edge col too. Interior pieces split the
                    # subtract between Pool (high half, TensorTensor at
                    # ~2.4ns/col but off the critical DVE stream) and DVE
                    # (low half, bf16 2x_1p 0.6ns/col) — this shaves ~5us
                    # off the DVE body. The globally last piece stays all
                    # on DVE, split in halves, so the tail drains fast.
                    pw = p1 - p0
                    last_piece = t == N_TILES - 1 and p0 == 0
                    if last_piece:
                        parts = [
                            ("v", p0 + pw // 2, p1, True),
                            ("v", p0, p0 + pw // 2, True),
                        ]
                    elif pw >= 1024:
                        # Pool takes the whole interior subtract (~4.9us at
                        # 2.4ns/col, inside PE's ~5.5us piece cadence): DVE
                        # then runs scans only and PE paces the body
                        parts = [("g", p0, p1, True)]
                    else:
                        parts = [("v", p0, p1, True)]
                    for eng, h0, h1, do_store in parts:
                        op_eng = nc.gpsimd if eng == "g" else nc.vector
                        op_eng.tensor_tensor(
                            out=r_t[t][:, h0:h1],
                            in0=ret_t[t][:, h0:h1],
                            in1=v_t[t][:, h0:h1],
                            op=sub,
                        )
                    cs = slice(p0, p1)
                    nc.sync.dma_start(out=ret[rows, cs], in_=ret_t[t][:, cs])
                    if last_piece:
                        nc.sync.dma_start(
                            out=adv[rows, p0 + pw // 2 : p1],
                            in_=r_t[t][:, p0 + pw // 2 : p1],
                        )
                        nc.sync.dma_start(
                            out=adv[rows, p0 : p0 + pw // 2],
                            in_=r_t[t][:, p0 : p0 + pw // 2],
                        )
                    else:
                        nc.sync.dma_start(out=adv[rows, cs], in_=r_t[t][:, cs])
    nc.finalize()
    return nc


def _get_nc():
    if "nc" not in _CACHE:
        _CACHE["nc"] = _build()
    return _CACHE["nc"]


def _run(rewards, values, next_values, **spmd_kwargs):
    """Shard over cores, run the Bass kernel, return BassKernelResults."""
    import ml_dtypes

    from concourse.bass_utils import run_bass_kernel_spmd

    bf16 = ml_dtypes.bfloat16
    nc = _get_nc()
    rewards = np.ascontiguousarray(rewards).astype(bf16)
    values = np.ascontiguousarray(values).astype(bf16)
    # B_{S-1} = r + c*(nv/lam) = r + gamma*nv: pre-scale so the kernel's
    # scan initial needs no edge handling
    nvs = np.ascontiguousarray(next_values, dtype=np.float32) / np.float32(LAM)
    ident = np.eye(P, dtype=bf16)
    gident = (np.eye(P) * (GAMMA * (1.0 - LAM))).astype(bf16)
    in_maps = []
    for c in range(N_CORES):
        sl = slice(c * ROWS, (c + 1) * ROWS)
        in_maps.append(
            {
                "rewards": rewards[sl],
                "values": values[sl],
                "next_values": nvs[sl],
                "ident": ident,
                "gident": gident,
            }
        )
    return run_bass_kernel_spmd(
        nc, in_maps, core_ids=list(range(N_CORES)), **spmd_kwargs
    )


def kernel(rewards, values, next_values):
    res = _run(rewards, values, next_values)
    advantages = np.concatenate(
        [res.results[c]["adv"] for c in range(N_CORES)], 0
    ).astype(np.float32)
    returns = np.concatenate(
        [res.results[c]["ret"] for c in range(N_CORES)], 0
    ).astype(np.float32)
    return advantages, returns
